# revision 29
# baseline (speedup 1.0000x reference)
"""LurieNet-k Trainium2 kernel.

Computes, from the raw parametrization tensors, the matrices
  C = UC @ SC @ VC^T,  B = UB @ SB @ VB^T,
  A = 0.5*UA @ SA @ UA^T + 0.5*YA  (SA = -(alpha_upp*I + GA))
entirely on device (matrix exponentials of skew matrices via
scaling-and-squaring Taylor), then runs the 511-step recurrence
  y  = C x + by
  x' = x + (0.01*A x + 0.01*B tanh(y) + 0.01*bx)
on a (128, 64) state shard per NeuronCore (batch data-parallel over the
8 cores), writing the full (b, t, n) trajectory.

Structure of the steady-state step (latency-, not throughput-bound):
the serial chain is tanh_t -> Q@th_t -> tanh_{t+1} (P-form:
y_{t+1} = P x_{t-1} + Q th_t + r with P = C + 0.01*C*A, Q = 0.01*C*B),
so everything else is arranged to stay off that chain:

* The fp32 state carry lives in a persistent PSUM bank accumulated
  directly by the A/B matmuls (never drained):
    bank_t = x_t - t*(0.01*bx)
  The only per-step DVE op reads it back rounded:
    xr_t = bf16(bank + t*0.01*bx)   [also the DMA'd output value]
  using a precomputed [N, TMAX] table of t*0.01*bx columns.
* P@xr_t is the PSUM accumulation *start* issued one full step before
  the Q@th stop, so only Q's fill+drain sits on the chain.
* Output transposes consume the bf16 xr slots (half the PE cost of
  fp32) and are delayed one step so their LDWEIGHTS never waits on the
  current step's DVE; the ACT drain copy casts bf16->fp32 for the DMA.

Precision: matmul operands are bf16 but the carry is exact fp32 (PSUM
accumulate), matching the baseline split-carry scheme; outputs are
bf16-rounded fp32 (adds ~1e-3 rms to the ~2e-3 rel err, gate is 2e-2).
"""

import sys

for _p in ("/opt/trn_rl_repo",):
    if _p not in sys.path:
        sys.path.insert(0, _p)

import numpy as np

import concourse.bass as bass
import concourse.mybir as mybir
import concourse.tile as tile
from concourse import bacc
from concourse import bass_isa
from concourse.bass import ds
from concourse.bass_utils import run_bass_kernel_spmd
from concourse.masks import make_identity, make_upper_triangular

F32 = mybir.dt.float32
F32R = mybir.dt.float32r
BF16 = mybir.dt.bfloat16
I32 = mybir.dt.int32
ALU = mybir.AluOpType
ACTF = mybir.ActivationFunctionType
AXIS = mybir.AxisListType

N = 128          # state dim
TMAX = 512       # time steps (including t=0)
BS = 512         # global batch
NCORES = 8
BSH = BS // NCORES   # 64 batch columns per core
STEP = 0.01
KTOP = 4

EXPM_SCAL = 3    # expm scaling: X = S / 2**EXPM_SCAL, then 3 squarings
EXPM_TERMS = 4   # Taylor terms in the Horner evaluation

PARAM_NAMES = [
    "ZA_Y", "ZA_U", "ZA_G", "ZB_U", "ZB_V", "ZB_S", "ZC_U", "ZC_V", "ZC_S",
]


def build_program(tmax=TMAX, tc_chunk=32, mdt=BF16):
    """Build the single-NeuronCore Bass program (run SPMD on all 8 cores).

    mdt: dtype of the recurrence matmul operands (weights, rounded state
    copy, tanh output). The fp32 state carry is exact regardless.
    """
    assert tmax % tc_chunk == 0 and tc_chunk % 2 == 0
    half = tc_chunk // 2
    nchunks = tmax // tc_chunk

    nc = bacc.Bacc(
        "TRN2",
        target_bir_lowering=False,
        debug=False,
        enable_asserts=False,
        num_devices=NCORES,
    )

    x0 = nc.dram_tensor("x0", [N, BSH], F32, kind="ExternalInput")
    zs = {
        name: nc.dram_tensor(name, [N, N], F32, kind="ExternalInput")
        for name in PARAM_NAMES
    }
    bx_d = nc.dram_tensor("bx", [N, 1], F32, kind="ExternalInput")
    by_d = nc.dram_tensor("by", [N, 1], F32, kind="ExternalInput")
    out = nc.dram_tensor("out", [BSH, tmax, N], F32, kind="ExternalOutput")

    with tile.TileContext(nc) as tc:
        with tc.tile_pool(name="const", bufs=1) as constp:
            # z-tensor loads issue first: the Sync engine serializes DMA
            # descriptor generation (~650ns each) and the expm phase is
            # gated on the five Z_U/Z_V inputs.
            zt = {}
            for name in ["ZC_U", "ZC_V", "ZB_U", "ZB_V", "ZA_U",
                         "ZC_S", "ZB_S", "ZA_G", "ZA_Y"]:
                zt[name] = constp.tile([N, N], F32, tag=name, name=f"z_{name}")
                nc.sync.dma_start(out=zt[name][:], in_=zs[name][:])

            ident = constp.tile([N, N], F32, tag="ident")
            make_identity(nc, ident[:])
            masku = constp.tile([N, N], F32, tag="masku")
            make_upper_triangular(nc, masku[:], val=1.0, diag=False)
            ident_r32 = constp.tile([N, N], F32R, tag="ident_r32")
            nc.vector.tensor_copy(ident_r32[:], ident[:])
            ident_bf = constp.tile([N, N], BF16, tag="ident_bf")
            nc.vector.tensor_copy(ident_bf[:], ident[:])

            by_c = constp.tile([N, 1], F32, tag="by")
            nc.sync.dma_start(out=by_c[:], in_=by_d[:])
            bx_c = constp.tile([N, 1], F32, tag="bxraw")
            nc.sync.dma_start(out=bx_c[:], in_=bx_d[:])
            bxp_c = constp.tile([N, 1], F32, tag="bxp")
            nc.vector.tensor_scalar_mul(bxp_c[:], bx_c[:], STEP)
            x0_c = constp.tile([N, BSH], F32, tag="x0c")
            nc.sync.dma_start(out=x0_c[:], in_=x0[:])

            # tb01[:, t] = t * 0.01 * bx  (bias column for the bank readout)
            tb01 = constp.tile([N, TMAX], F32, tag="tb01")

            # ------- setup phase: expm's + weight assembly -------
            # Recurrence uses the P-form to split the serial chain in two:
            #   y_t  = P x_{t-2} + Q th_{t-1} + r,  th_t = tanh(y_t)
            #   x_t  = x_{t-1} + (0.01A) x_{t-1} + (0.01B) th_t + 0.01 bx
            # with P = C + 0.01 C A, Q = 0.01 C B, r = 0.01 C bx + by.
            PTm = constp.tile([N, N], mdt, tag="PTm")      # P^T
            QTm = constp.tile([N, N], mdt, tag="QTm")      # Q^T
            A01Tm = constp.tile([N, N], mdt, tag="A01Tm")  # (0.01 A)^T
            BpTm = constp.tile([N, N], mdt, tag="BpTm")    # (0.01 B)^T
            CTf32 = constp.tile([N, N], F32, tag="CTf32")  # C^T fp32
            r_c = constp.tile([N, 1], F32, tag="rc")       # 0.01 C bx + by
            # second-level P-form: y_{t+1} = P2 x_{t-2} + PB th_{t-1}
            #                              + Q th_t + r2
            # pushes the state operand two steps back so nothing in the PE
            # pipe is gated by the just-finished bank readout.
            P2Tm = constp.tile([N, N], mdt, tag="P2Tm")    # (P(I+0.01A))^T
            PBTm = constp.tile([N, N], mdt, tag="PBTm")    # (0.01 P B)^T
            r2_c = constp.tile([N, 1], F32, tag="r2c")     # r + 0.01 P bx

            with (
                tc.tile_pool(name="work", bufs=2) as wp,
                tc.tile_pool(name="eres", bufs=1) as ep,
                tc.tile_pool(name="small", bufs=1) as sp,
                tc.tile_pool(name="pss", bufs=4, space="PSUM") as psp,
            ):
                tbi = wp.tile([N, TMAX], I32, tag="tbi")
                nc.gpsimd.iota(
                    tbi[:], pattern=[[1, TMAX]], base=0, channel_multiplier=0
                )
                tbf = wp.tile([N, TMAX], F32, tag="tbf")
                nc.vector.tensor_copy(tbf[:], tbi[:])
                nc.vector.tensor_scalar_mul(tb01[:], tbf[:], bxp_c[:])

                def expm_batch(specs):
                    """Interleaved expm(skew(Z))^T for all matrices at once.

                    Maintains the (T, T^T) pair through Horner + squaring so
                    no PE transposes are needed: with negX = X^T = -X,
                      X @ T     = matmul(lhsT=negX, rhs=T)
                      T^T @ X^T = matmul(lhsT=T,    rhs=negX)
                    The five chains are advanced stage-by-stage so PE/DVE/ACT
                    work from different chains overlaps (a single sequential
                    chain is latency-bound on the MM->STT->MM dependency).
                    """
                    scal = 1.0 / (2.0 ** EXPM_SCAL)
                    negx = {}
                    t_cur = {}
                    tt_cur = {}
                    for z_tile, tag in specs:
                        us = wp.tile([N, N], F32R, tag="us_r", name=f"us_{tag}")
                        nc.vector.scalar_tensor_tensor(
                            us[:], z_tile[:], scal, masku[:],
                            op0=ALU.mult, op1=ALU.mult,
                        )
                        pst = psp.tile([N, N], F32R, tag="ps", bufs=8,
                                       name=f"pst_{tag}")
                        nc.tensor.transpose(pst[:], us[:], ident_r32[:])
                        nx = wp.tile([N, N], F32R, tag=f"negx_{tag}", bufs=1,
                                     name=f"negx_{tag}")
                        nc.vector.scalar_tensor_tensor(
                            nx[:], pst[:], 1.0, us[:],
                            op0=ALU.mult, op1=ALU.subtract,
                        )
                        negx[tag] = nx
                        t_cur[tag] = ident_r32
                        tt_cur[tag] = ident_r32
                    # Horner advances only T (the T^T pair is rebuilt by a
                    # PE transpose afterwards -- halves the vector-op load,
                    # which bounds this phase).
                    for j in range(EXPM_TERMS, 0, -1):
                        for _, tag in specs:
                            psa = psp.tile([N, N], F32, tag="ps", bufs=8)
                            nc.tensor.matmul(
                                psa[:], negx[tag][:], t_cur[tag][:],
                                start=True, stop=True,
                            )
                            t_new = wp.tile([N, N], F32R, tag=f"T_{tag}",
                                            bufs=2, name=f"T_{tag}")
                            nc.vector.scalar_tensor_tensor(
                                t_new[:], psa[:], 1.0 / j, ident_r32[:],
                                op0=ALU.mult, op1=ALU.add,
                            )
                            t_cur[tag] = t_new
                    for _, tag in specs:
                        pst = psp.tile([N, N], F32R, tag="ps", bufs=8,
                                       name=f"ptt_{tag}")
                        nc.tensor.transpose(pst[:], t_cur[tag][:], ident_r32[:])
                        tt_new = wp.tile([N, N], F32R, tag=f"TT_{tag}",
                                         bufs=2, name=f"TT_{tag}")
                        nc.scalar.copy(tt_new[:], pst[:])
                        tt_cur[tag] = tt_new
                    for si in range(EXPM_SCAL):
                        # T of the final squaring is never consumed: skip its
                        # matmul/copy but keep the pool allocations so the
                        # psum rotation (and hence bank placement) is fixed.
                        last = si == EXPM_SCAL - 1
                        for _, tag in specs:
                            psa = psp.tile([N, N], F32, tag="ps", bufs=8)
                            psb = psp.tile([N, N], F32, tag="ps", bufs=8)
                            if not last:
                                nc.tensor.matmul(
                                    psa[:], tt_cur[tag][:], t_cur[tag][:],
                                    start=True, stop=True,
                                )
                            nc.tensor.matmul(
                                psb[:], t_cur[tag][:], tt_cur[tag][:],
                                start=True, stop=True,
                            )
                            tt_new = wp.tile([N, N], F32R, tag=f"TT_{tag}",
                                             bufs=2, name=f"TT_{tag}")
                            nc.scalar.copy(tt_new[:], psb[:])
                            tt_cur[tag] = tt_new
                            if not last:
                                t_new = wp.tile([N, N], F32R, tag=f"T_{tag}",
                                                bufs=2, name=f"T_{tag}")
                                nc.vector.tensor_copy(t_new[:], psa[:])
                                t_cur[tag] = t_new
                    return tt_cur

                eres = expm_batch([
                    (zt["ZC_U"], "UCT"), (zt["ZC_V"], "VCT"),
                    (zt["ZB_U"], "UBT"), (zt["ZB_V"], "VBT"),
                    (zt["ZA_U"], "UAT"),
                ])
                uct, vct = eres["UCT"], eres["VCT"]
                ubt, vbt = eres["UBT"], eres["VBT"]
                uat = eres["UAT"]

                def absdiag_col(z_tile, tag):
                    tmp = wp.tile([N, N], F32, tag="us")
                    nc.vector.tensor_mul(tmp[:], z_tile[:], ident[:])
                    col = sp.tile([N, 1], F32, tag=tag, name=f"col_{tag}")
                    nc.vector.tensor_reduce(
                        col[:], tmp[:], AXIS.X, ALU.add, apply_absolute_value=True
                    )
                    return col

                dc_col = absdiag_col(zt["ZC_S"], "dc")   # |diag(ZC_S)|
                db_col = absdiag_col(zt["ZB_S"], "db")   # |diag(ZB_S)|
                ga_col = absdiag_col(zt["ZA_G"], "ga")   # |diag(ZA_G)|

                sb01 = sp.tile([N, 1], F32, tag="sb01")
                nc.vector.tensor_scalar_mul(sb01[:], db_col[:], STEP)

                # C^T = VC @ (SC @ UC^T)
                p1 = wp.tile([N, N], F32R, tag="us_r", name="p1")
                nc.vector.tensor_scalar_mul(p1[:], uct[:], dc_col[:])
                psa = psp.tile([N, N], F32, tag="ps", bufs=8)
                nc.tensor.matmul(psa[:], vct[:], p1[:], start=True, stop=True)
                nc.vector.tensor_copy(CTf32[:], psa[:])

                # (0.01 B)^T = VB @ (0.01 SB @ UB^T)
                p2 = wp.tile([N, N], F32R, tag="us_r", name="p2")
                nc.vector.tensor_scalar_mul(p2[:], ubt[:], sb01[:])
                psb = psp.tile([N, N], F32, tag="ps", bufs=8)
                nc.tensor.matmul(psb[:], vbt[:], p2[:], start=True, stop=True)
                nc.vector.tensor_copy(BpTm[:], psb[:])
                # untransposed 0.01 B = UB @ (0.01 SB @ VB^T)
                p2b = wp.tile([N, N], F32R, tag="us_r", name="p2b")
                nc.vector.tensor_scalar_mul(p2b[:], vbt[:], sb01[:])
                psb2 = psp.tile([N, N], F32, tag="ps", bufs=8)
                nc.tensor.matmul(psb2[:], ubt[:], p2b[:], start=True, stop=True)
                bp_un = constp.tile([N, N], F32, tag="Bpun")
                nc.vector.tensor_copy(bp_un[:], psb2[:])

                # top-4: alpha = sqrt(sum_i (b_i c_i)^2), b/c sorted desc.
                bwork = sp.tile([N, 1], F32, tag="bwork")
                cwork = sp.tile([N, 1], F32, tag="cwork")
                nc.vector.tensor_copy(bwork[:], db_col[:])
                nc.vector.tensor_copy(cwork[:], dc_col[:])
                acc = sp.tile([N, 1], F32, tag="acc")
                nc.vector.memset(acc[:], 0.0)
                bmax = sp.tile([N, 1], F32, tag="bmax")
                cmax = sp.tile([N, 1], F32, tag="cmax")
                prod = sp.tile([N, 1], F32, tag="prod")
                gmask = sp.tile([N, 1], F32, tag="gmask")
                tdrop = sp.tile([N, 1], F32, tag="tdrop")
                for i in range(KTOP):
                    nc.gpsimd.partition_all_reduce(
                        bmax[:], bwork[:], N, bass_isa.ReduceOp.max
                    )
                    nc.gpsimd.partition_all_reduce(
                        cmax[:], cwork[:], N, bass_isa.ReduceOp.max
                    )
                    nc.vector.tensor_mul(prod[:], bmax[:], cmax[:])
                    nc.vector.tensor_mul(prod[:], prod[:], prod[:])
                    nc.vector.tensor_add(acc[:], acc[:], prod[:])
                    if i < KTOP - 1:
                        # zero out the extracted max (values all > 0)
                        nc.vector.tensor_single_scalar(
                            gmask[:], bwork[:], bmax[:], ALU.is_ge
                        )
                        nc.vector.tensor_mul(tdrop[:], bwork[:], gmask[:])
                        nc.vector.tensor_sub(bwork[:], bwork[:], tdrop[:])
                        nc.vector.tensor_single_scalar(
                            gmask[:], cwork[:], cmax[:], ALU.is_ge
                        )
                        nc.vector.tensor_mul(tdrop[:], cwork[:], gmask[:])
                        nc.vector.tensor_sub(cwork[:], cwork[:], tdrop[:])
                alpha = sp.tile([N, 1], F32, tag="alpha")
                nc.scalar.activation(alpha[:], acc[:], ACTF.Sqrt)

                # sa05 = -0.5*(alpha + gA)  (per-partition row scale of UA^T)
                sa05 = sp.tile([N, 1], F32, tag="sa05")
                nc.vector.tensor_scalar(
                    sa05[:], ga_col[:], alpha[:], -0.5, op0=ALU.add, op1=ALU.mult
                )
                # M = UA @ (sa05 * UA^T) = 0.5*UA SA UA^T (symmetric)
                p3 = wp.tile([N, N], F32R, tag="us_r", name="p3")
                nc.vector.tensor_scalar_mul(p3[:], uat[:], sa05[:])
                psm = psp.tile([N, N], F32, tag="ps", bufs=8)
                nc.tensor.matmul(psm[:], uat[:], p3[:], start=True, stop=True)
                # YA = Uy - Uy^T; q2 = -0.005*YA
                uy = wp.tile([N, N], F32, tag="us")
                nc.vector.tensor_mul(uy[:], zt["ZA_Y"][:], masku[:])
                pst2 = psp.tile([N, N], F32, tag="ps", bufs=8)
                nc.tensor.transpose(pst2[:], uy[:], ident[:])
                nc.vector.tensor_scalar_mul(uy[:], uy[:], 0.5 * STEP)
                q2 = wp.tile([N, N], F32, tag="T")
                nc.vector.scalar_tensor_tensor(
                    q2[:], pst2[:], 0.5 * STEP, uy[:], op0=ALU.mult, op1=ALU.subtract
                )
                # (0.01 A)^T = 0.01*M + q2 ; untransposed 0.01 A = 0.01*M - q2
                nc.vector.scalar_tensor_tensor(
                    A01Tm[:], psm[:], STEP, q2[:], op0=ALU.mult, op1=ALU.add
                )
                a01_un = constp.tile([N, N], F32, tag="A01un")
                nc.vector.scalar_tensor_tensor(
                    a01_un[:], psm[:], STEP, q2[:], op0=ALU.mult, op1=ALU.subtract
                )

                # P^T = C^T + (0.01 A)^T C^T ;  Q^T = (0.01 B)^T C^T
                psw = psp.tile([N, N], F32, tag="ps", bufs=8)
                nc.tensor.matmul(psw[:], a01_un[:], CTf32[:], start=True, stop=True)
                ptf = constp.tile([N, N], F32, tag="PTf32")
                nc.vector.scalar_tensor_tensor(
                    ptf[:], psw[:], 1.0, CTf32[:], op0=ALU.mult, op1=ALU.add
                )
                nc.vector.tensor_copy(PTm[:], ptf[:])
                psq = psp.tile([N, N], F32, tag="ps", bufs=8)
                nc.tensor.matmul(psq[:], bp_un[:], CTf32[:], start=True, stop=True)
                nc.vector.tensor_copy(QTm[:], psq[:])


                # r = 0.01 C bx + by ;  r2 = r + 0.01 P bx
                psr = psp.tile([N, 1], F32, tag="ps", bufs=8, name="psr")
                nc.tensor.matmul(psr[:], CTf32[:], bxp_c[:], start=True, stop=True)
                nc.vector.scalar_tensor_tensor(
                    r_c[:], psr[:], 1.0, by_c[:], op0=ALU.mult, op1=ALU.add
                )

            # ------- recurrence (PSUM-resident carry) -------
            with (
                tc.tile_pool(name="xrbuf", bufs=2) as xbufp,
                tc.tile_pool(name="stage", bufs=2) as stagep,
                tc.tile_pool(name="th", bufs=3) as thp,
                tc.tile_pool(name="bank", bufs=1, space="PSUM") as bankp,
                tc.tile_pool(name="psy", bufs=1, space="PSUM") as psyp,
                tc.tile_pool(name="pstr", bufs=2, space="PSUM") as pstrp,
            ):
                xbank = bankp.tile([N, BSH], F32, tag="xbank")
                pscr = bankp.tile([N, N], F32, tag="pscr")
                ppA = psyp.tile([N, BSH], F32, tag="ppA")
                ppB = psyp.tile([N, BSH], F32, tag="ppB")
                pp = [ppA, ppB]
                psy0 = psyp.tile([N, BSH], F32, tag="psy0")

                # boot: bank = x0 (exact fp32); th_1 = tanh(C x0 + by)
                nc.vector.tensor_copy(xbank[:], x0_c[:])
                nc.tensor.matmul(psy0[:], CTf32[:], x0_c[:], start=True, stop=True)
                th_prev = thp.tile([N, BSH], mdt, tag="th", name="th_init")
                nc.scalar.activation(
                    th_prev[:], psy0[:], ACTF.Tanh, bias=by_c[:], scale=1.0
                )

                # finish weight assembly AFTER the boot emission: P2/PB are
                # first consumed at t=2 (~1.3us after the first tanh),
                # so these three matmuls overlap the first steps
                # instead of delaying them. (Kept out of the setup
                # psum pool so its rotation stays within its 8 banks:
                # rotation stays within its 8 banks:
                # P2^T = P^T + (0.01A)^T P^T ; PB^T = (0.01B)^T P^T
                nc.tensor.matmul(pscr[:], a01_un[:], ptf[:], start=True, stop=True)
                nc.vector.scalar_tensor_tensor(
                    P2Tm[:], pscr[:], 1.0, ptf[:], op0=ALU.mult, op1=ALU.add
                )
                nc.tensor.matmul(pscr[:], bp_un[:], ptf[:], start=True, stop=True)
                nc.vector.tensor_copy(PBTm[:], pscr[:])
                # r2 = r + 0.01 P bx
                nc.tensor.matmul(
                    pscr[:, 0:1], ptf[:], bxp_c[:], start=True, stop=True
                )
                nc.vector.scalar_tensor_tensor(
                    r2_c[:], pscr[:, 0:1], 1.0, r_c[:], op0=ALU.mult, op1=ALU.add
                )

                # xr column slot for local step s: pairs (i, i+half) are
                # adjacent so the PE transpose reads one contiguous block
                # (matmul weight APs must have a single free dim).
                def slot(s):
                    return 2 * (s % half) + (s // half)

                pstr_box = [None]

                def emit_transpose(j, xb, st):
                    # pair j = slots (2j, 2j+1) = chunk steps (j, j+half)
                    if j % 2 == 0:
                        pstr_box[0] = pstrp.tile(
                            [128, 2 * N], mdt, tag="pstr", name="pstr"
                        )
                    pstr = pstr_box[0]
                    nc.tensor.transpose(
                        pstr[:, ds((j % 2) * N, N)],
                        xb[:, ds(2 * j * BSH, 2 * BSH)],
                        ident_bf[:],
                    )
                    if j % 2 == 1:
                        # one DVE copy drains two pair-transposes, casting
                        # bf16 -> fp32 for the output DMA (kept off the ACT
                        # engine, which must stay free for the chain tanh).
                        nc.vector.tensor_copy(
                            st[:, ds((j - 1) * N, 2 * N)], pstr[:, 0:2 * N]
                        )

                qn = max(half // 4, 1)

                def emit_dma(c, st, q0s):
                    for h in range(2):
                        for q0 in q0s:
                            t0 = c * tc_chunk + h * half + q0
                            dram_ap = out[:, t0:t0 + qn, :].rearrange(
                                "b i n -> b (i n)"
                            )
                            nc.sync.dma_start(
                                out=dram_ap,
                                in_=st[h * 64:(h + 1) * 64, ds(q0 * N, qn * N)],
                            )

                # xr_m1/xr_m2: the bank readouts from one/two iterations ago;
                # th_prev/th_prev2: tanh outputs th_t / th_{t-1}.
                xr_m1 = xr_m2 = None
                th_prev2 = None
                xb_prev = st_prev = None
                for c in range(nchunks):
                    xb = xbufp.tile([N, tc_chunk * BSH], mdt, tag="xb")
                    st = stagep.tile([128, half * N], F32, tag="st")
                    for s in range(tc_chunk):
                        t = c * tc_chunk + s
                        if t == 1:
                            # boot group for y_2 = P x_0 + Q th_1 + r
                            nc.tensor.matmul(
                                pp[1][:], PTm[:], xr_m1,
                                start=True, stop=False,
                            )
                        elif 2 <= t <= tmax - 2:
                            # y_{t+1} = P2 x_{t-2} + PB th_{t-1} + Q th_t + r2
                            # (both operands old: exec immediately, pipe clear
                            # well before the th-gated Q slot)
                            nc.tensor.matmul(
                                pp[t % 2][:], P2Tm[:], xr_m2,
                                start=True, stop=False,
                            )
                            nc.tensor.matmul(
                                pp[t % 2][:], PBTm[:], th_prev2[:],
                                start=False, stop=False,
                            )
                        if t > 0:
                            # th-chain critical: Q@th_t completes y_{t+1}
                            if t <= tmax - 2:
                                nc.tensor.matmul(
                                    pp[t % 2][:], QTm[:], th_prev[:],
                                    start=False, stop=True,
                                )
                            # x-chain: bank += 0.01B th_t  (bank: x_{t-1}->x_t)
                            nc.tensor.matmul(
                                xbank[:], BpTm[:], th_prev[:],
                                start=False, stop=False, skip_group_check=True,
                            )
                            # bank += 0.01A xr_{t-1} (-> x_t); after B so its
                            # fill never delays Q
                            nc.tensor.matmul(
                                xbank[:], A01Tm[:], xr_m1,
                                start=False, stop=False, skip_group_check=True,
                            )
                        if 0 < t <= tmax - 2:
                            th_new = thp.tile([N, BSH], mdt, tag="th")
                            nc.scalar.activation(
                                th_new[:], pp[t % 2][:], ACTF.Tanh,
                                bias=(r_c[:] if t == 1 else r2_c[:]), scale=1.0,
                            )
                        else:
                            th_new = th_prev
                        # output + rounded state: xr_t = bf16(bank + t*0.01*bx)
                        xr_ap = xb[:, ds(slot(s) * BSH, BSH)]
                        if t == 0:
                            nc.vector.tensor_copy(xr_ap, x0_c[:])
                        else:
                            nc.vector.tensor_scalar_add(
                                xr_ap, xbank[:], tb01[:, t:t + 1]
                            )
                        xr_m1, xr_m2 = xr_ap, xr_m1
                        # delayed output transposes (second slot of pair j was
                        # written last iteration, so the LDWEIGHTS sem is long
                        # satisfied); emitted after xr so the DVE services the
                        # chain-relevant xr readout before the drain copy.
                        # Pair half-1 is carried into the next chunk.
                        if s > half:
                            emit_transpose(s - half - 1, xb, st)
                        elif s == 0 and c > 0:
                            emit_transpose(half - 1, xb_prev, st_prev)
                            emit_dma(c - 1, st_prev, [half - qn])
                        th_prev2, th_prev = th_prev, th_new

                    emit_dma(c, st, list(range(0, half - qn, qn)))
                    xb_prev, st_prev = xb, st

                # drain the carried last pair + final DMA group
                emit_transpose(half - 1, xb_prev, st_prev)
                emit_dma(nchunks - 1, st_prev, [half - qn])

    nc.compile()
    return nc


_CACHED = {}


def _get_program(tmax=TMAX, tc_chunk=32, mdt=BF16):
    key = (tmax, tc_chunk, str(mdt))
    if key not in _CACHED:
        _CACHED[key] = build_program(tmax, tc_chunk, mdt)
    return _CACHED[key]


def make_in_maps(inputs, tmax=TMAX):
    X0 = np.ascontiguousarray(np.asarray(inputs["X0"], dtype=np.float32))
    base = {
        name: np.ascontiguousarray(np.asarray(inputs[name], dtype=np.float32))
        for name in PARAM_NAMES
    }
    base["bx"] = np.ascontiguousarray(
        np.asarray(inputs["bx"], dtype=np.float32).reshape(N, 1)
    )
    base["by"] = np.ascontiguousarray(
        np.asarray(inputs["by"], dtype=np.float32).reshape(N, 1)
    )
    in_maps = []
    for c in range(NCORES):
        m = dict(base)
        m["x0"] = np.ascontiguousarray(X0[c * BSH:(c + 1) * BSH].T)
        in_maps.append(m)
    return in_maps


def run_spmd(inputs, tmax=TMAX, tc_chunk=32, trace=False, tmpdir=None, mdt=BF16):
    nc = _get_program(tmax, tc_chunk, mdt)
    in_maps = make_in_maps(inputs, tmax)
    res = run_bass_kernel_spmd(
        nc, in_maps, list(range(NCORES)), trace=trace, tmpdir=tmpdir
    )
    outs = [res.results[c]["out"] for c in range(NCORES)]
    full = np.concatenate(outs, axis=0)
    return full, res


def kernel(**inputs):
    full, _ = run_spmd(inputs)
    return full
